# revision 1
# baseline (speedup 1.0000x reference)
"""Trainium2 Bass kernel for nn_AttentionLayer (B=4, S=2048, H=16, DH=64).

Sharding: 8 cores = 4 batches x 2 head-halves. Core c handles batch c//2,
heads (c%2)*8 .. (c%2)*8+8 (i.e. 512 of the 1024 QKV columns).

Per-core device program (SPMD, same program on all cores, different inputs):
  inputs (pre-laid-out on host):
    xT  [1024, 2048]  = x[b].T           (contraction dim on partitions)
    wq/wk/wv [1024, 512]                 (column slice for this core's heads)
    bq/bk/bv [512]
  output:
    out [512, 2048] = attention ctx for this core's 8 heads, transposed
                      (head*64+dh on rows, seq on cols); host transposes back.

Structure (single TileContext; all matmuls fp32r = full PE rate, ~1e-4 rel
error; Tile schedules by dependency + priority):
  - Priority bands: every attention-unit instruction outranks the QKV/V
    "filler" work, so the exp pipeline never starves while projections
    gap-fill the PE between attention matmuls.
  - V pass: V = x@Wv (PE) + bv (DVE add) -> v_aug [128, 8, 65] x16 seq tiles
    (col 64 = ones; the 65th PV output row accumulates the softmax
    denominator for free).
  - Per head-pair m: QT/KT = (x@W)^T on PE + bias via tensor_scalar_add on
    the PSUM->SBUF copy (weights streamed per pair, x re-streamed from HBM;
    Q chunks 2,3 deferred past qg0 to rebalance PE). Then attention units
    (m, qgroup of 1024 q, head):
      per k-block kb: scoresT [128,1024] = KT_chunk.T @ QT (PE, K=64);
                      E = exp(scoresT/8) (ACT, PSUM->SBUF, f32r out);
                      ctxT [65,512]x2 += v_aug.T @ E (PE, PSUM-accumulated)
      then per 512-q half: copy ctx to SBUF (frees PSUM), reciprocal of row
      64 (DVE), partition-broadcast (GPSIMD), multiply (DVE), DMA out.
  Emission is software-pipelined two scores-blocks ahead of PV so the next
  exp input always wins the PE race against the previous PV; the first
  attention unit is split 512-wide to start exp as soon as QK chunk 0 lands.
  The final unit normalizes directly from PSUM (no successor needs its ctx
  slot, so the early-release copy is skipped there).
  PSUM: scores 2x2 banks + ctx 2x1 + qkv 2 = 8. Modeled (TimelineSim)
  per-core time ~332 us; ACT (exp) busy ~267 us, PE busy ~305 us.
"""

import numpy as np

B, S, H, DH = 4, 2048, 16, 64
D = H * DH  # 1024
NCORES = 8
COLS = 512  # qkv columns per core (8 heads)
NPAIR = 4  # head pairs per core
NKB = S // 128  # 16 k-blocks
QG = 1024  # q-group width
NQG = S // QG  # 2
XC = 512  # x streaming chunk (seq cols)
NXC = S // XC  # 4
INV_SQRT_DH = 1.0 / 8.0

_CACHE = {}


def _build():
    import concourse.mybir as mybir
    import concourse.tile as tile
    from concourse import bacc

    f32 = mybir.dt.float32
    f32r = mybir.dt.float32r
    Exp = mybir.ActivationFunctionType.Exp

    nc = bacc.Bacc(
        "TRN2",
        target_bir_lowering=False,
        debug=False,
        enable_asserts=False,
        num_devices=NCORES,
    )

    xT_d = nc.dram_tensor("xT", [D, S], f32r, kind="ExternalInput").ap()
    wq_d = nc.dram_tensor("wq", [D, COLS], f32r, kind="ExternalInput").ap()
    wk_d = nc.dram_tensor("wk", [D, COLS], f32r, kind="ExternalInput").ap()
    wv_d = nc.dram_tensor("wv", [D, COLS], f32r, kind="ExternalInput").ap()
    bq_d = nc.dram_tensor("bq", [COLS], f32, kind="ExternalInput").ap()
    bk_d = nc.dram_tensor("bk", [COLS], f32, kind="ExternalInput").ap()
    bv_d = nc.dram_tensor("bv", [COLS], f32, kind="ExternalInput").ap()
    out_d = nc.dram_tensor("out", [COLS, S], f32, kind="ExternalOutput").ap()

    with tile.TileContext(nc) as tc:
        with (
            tc.tile_pool(name="consts", bufs=1) as consts,
            tc.tile_pool(name="vpool", bufs=1) as vpool,
            tc.tile_pool(name="wvpool", bufs=1) as wvpool,
            tc.tile_pool(name="wqk", bufs=2) as wqk,
            tc.tile_pool(name="xpool", bufs=2) as xpool,
            tc.tile_pool(name="qkt", bufs=2) as qkt,
            tc.tile_pool(name="epool", bufs=12) as epool,
            tc.tile_pool(name="opool", bufs=2) as opool,
            tc.tile_pool(name="psum", bufs=1, space="PSUM") as psum,
        ):
            # ---- constants, ACT table preload ----
            bq_t = consts.tile([128, NPAIR], f32)
            bk_t = consts.tile([128, NPAIR], f32)
            bv_s = consts.tile([1, COLS], f32)
            bvb = consts.tile([128, COLS], f32)
            nc.gpsimd.dma_start(out=bq_t, in_=bq_d.rearrange("(m p) -> p m", p=128))
            nc.gpsimd.dma_start(out=bk_t, in_=bk_d.rearrange("(m p) -> p m", p=128))
            nc.gpsimd.dma_start(out=bv_s, in_=bv_d[None, :])
            nc.gpsimd.partition_broadcast(bvb, bv_s)
            warm = consts.tile([1, 1], f32)
            nc.vector.memset(warm, 0.0)
            nc.scalar.activation(warm, warm, Exp)  # pull ACT table load early

            vt = [vpool.tile([128, 8, 65], f32r, name=f"vt{i}") for i in range(NKB)]
            for i in range(NKB):
                nc.vector.memset(vt[i][:, :, 64:65].bitcast(f32), 1.0)

            wv = wvpool.tile([128, 8, COLS], f32r, name="wv")

            def load_wv():
                nc.sync.dma_start(
                    out=wv, in_=wv_d.rearrange("(j p) c -> p j c", p=128)
                )

            def load_x_chunk(c, wpair=None, eng=None):
                if wpair is not None:
                    load_w_dma(*wpair)
                if eng is None:
                    eng = nc.sync
                xt = xpool.tile([128, 8, XC], f32r, name="xt", tag="xt")
                for j in range(8):
                    eng.dma_start(
                        out=xt[:, j, :],
                        in_=xT_d[j * 128 : (j + 1) * 128, c * XC : (c + 1) * XC],
                    )
                return xt

            def v_pass(chunks):
                for c in chunks:
                    xt = load_x_chunk(c)
                    for i in range(XC // 128):
                        it = c * (XC // 128) + i
                        ps = psum.tile([128, 512], f32, tag="p1", bufs=2)
                        for j in range(8):
                            nc.tensor.matmul(
                                ps,
                                lhsT=xt[:, j, i * 128 : (i + 1) * 128],
                                rhs=wv[:, j, :],
                                start=(j == 0),
                                stop=(j == 7),
                            )
                        nc.vector.tensor_add(
                            vt[it][:, :, 0:64],
                            ps.rearrange("p (h d) -> p h d", h=8),
                            bvb.rearrange("p (h d) -> p h d", h=8),
                        )

            def load_w_dma(wqm, wkm, m):
                nc.sync.dma_start(
                    out=wkm,
                    in_=wk_d[:, m * 128 : (m + 1) * 128].rearrange(
                        "(j p) c -> p j c", p=128
                    ),
                )
                nc.sync.dma_start(
                    out=wqm,
                    in_=wq_d[:, m * 128 : (m + 1) * 128].rearrange(
                        "(j p) c -> p j c", p=128
                    ),
                )

            def load_w_pair(m, defer_dma=False):
                wqm = wqk.tile([128, 8, 128], f32r, name="wqm", tag="wqm")
                wkm = wqk.tile([128, 8, 128], f32r, name="wkm", tag="wkm")
                if not defer_dma:
                    load_w_dma(wqm, wkm, m)
                return wqm, wkm

            def qk_chunk(m, c, xt, wqm, wkm, qt, kt, projs=("k", "q")):
                pairs = {"q": (wqm, bq_t, qt), "k": (wkm, bk_t, kt)}
                for w, bias, dst in (pairs[p] for p in projs):
                    ps = psum.tile([128, 512], f32, tag="p1", bufs=2)
                    for j in range(8):
                        nc.tensor.matmul(
                            ps,
                            lhsT=w[:, j, :],
                            rhs=xt[:, j, :],
                            start=(j == 0),
                            stop=(j == 7),
                        )
                    nc.vector.tensor_scalar_add(
                        dst[:, c * XC : (c + 1) * XC], ps, bias[:, m : m + 1]
                    )

            def v_chunk(c, xt):
                for i in range(XC // 128):
                    it = c * (XC // 128) + i
                    ps = psum.tile([128, 512], f32, tag="p1", bufs=2)
                    for j in range(8):
                        nc.tensor.matmul(
                            ps,
                            lhsT=xt[:, j, i * 128 : (i + 1) * 128],
                            rhs=wv[:, j, :],
                            start=(j == 0),
                            stop=(j == 7),
                        )
                    nc.vector.tensor_add(
                        vt[it][:, :, 0:64],
                        ps.rearrange("p (h d) -> p h d", h=8),
                        bvb.rearrange("p (h d) -> p h d", h=8),
                    )

            def emit_attention_unit(m, qg, h, q0, p0, head, qt, kt, qw=QG, last=False):
                nq = qw // 512
                ctx = [
                    psum.tile([65, 512], f32, tag="ctx", bufs=2, name=f"ctx{qq}")
                    for qq in range(nq)
                ]

                def scores(kb):
                    sc = psum.tile([128, qw], f32, tag="sc", bufs=2)
                    for qq in range(nq):
                        nc.tensor.matmul(
                            sc[:, qq * 512 : (qq + 1) * 512],
                            lhsT=kt[p0 : p0 + 64, kb * 128 : (kb + 1) * 128],
                            rhs=qt[
                                p0 : p0 + 64,
                                q0 + qq * 512 : q0 + (qq + 1) * 512,
                            ],
                            start=True,
                            stop=True,
                        )
                    return sc

                # software-pipelined emission, two scores ahead: at exp(kb)'s
                # end both PV(kb) and scores(kb+2) become ready on PE; the
                # scores must win that race (higher priority = emitted
                # earlier) or exp(kb+2) slips past exp(kb+1)'s window.
                scs = [scores(0), scores(1)]
                for kb in range(NKB):
                    ee = epool.tile([128, qw], f32r, tag="e")
                    nc.scalar.activation(ee, scs[kb % 2], Exp, scale=INV_SQRT_DH)
                    if kb < NKB - 2:
                        scs[kb % 2] = scores(kb + 2)
                    for qq in range(nq):
                        nc.tensor.matmul(
                            ctx[qq],
                            lhsT=vt[kb][:, head, :],
                            rhs=ee[:, qq * 512 : (qq + 1) * 512],
                            start=(kb == 0),
                            stop=(kb == NKB - 1),
                        )
                for qq in range(nq):
                    if last:
                        src_t = ctx[qq]
                    else:
                        cs = opool.tile([65, 512], f32, tag="cs")
                        nc.vector.tensor_copy(cs, ctx[qq])
                        src_t = cs
                    rr = opool.tile([1, 512], f32, tag="r")
                    nc.vector.reciprocal(rr, src_t[64:65, :])
                    rb = opool.tile([64, 512], f32, tag="rb")
                    nc.gpsimd.partition_broadcast(rb, rr)
                    ob = opool.tile([64, 512], f32, tag="o")
                    nc.vector.tensor_mul(ob, src_t[0:64, :], rb)
                    nc.sync.dma_start(
                        out=out_d[
                            head * 64 : (head + 1) * 64,
                            q0 + qq * 512 : q0 + (qq + 1) * 512,
                        ],
                        in_=ob,
                    )

            # ---- banded priorities: attention preferred, QKV/V fill gaps ----
            from contextlib import contextmanager

            base = tc.cur_priority + 50
            att_cur = [base]
            fill_cur = [base + 6000]

            @contextmanager
            def band(cursor):
                off = tc.cur_priority - cursor[0]
                with tc.high_priority(offset=off):
                    yield
                    cursor[0] = tc.cur_priority

            # ---- per pair: QKV (filler band) then attention (att band) ----
            for m in range(NPAIR):
                with band(fill_cur):
                    wqm, wkm = load_w_pair(m, defer_dma=(m == 0))
                    qt = qkt.tile([128, S], f32r, name=f"qt{m}", tag="qt")
                    kt = qkt.tile([128, S], f32r, name=f"kt{m}", tag="kt")
                    xts = {}
                    if m == 0:
                        xts[0] = load_x_chunk(0, wpair=(wqm, wkm, m))
                        xts[1] = load_x_chunk(1)
                        qk_chunk(m, 0, xts[0], wqm, wkm, qt, kt)
                        qk_chunk(m, 1, xts[1], wqm, wkm, qt, kt)
                        load_wv()
                        v_chunk(0, xts[0])
                        v_chunk(1, xts[1])
                        for c in (2, 3):
                            xts[c] = load_x_chunk(c)
                            qk_chunk(m, c, xts[c], wqm, wkm, qt, kt)
                            v_chunk(c, xts[c])
                    else:
                        for c in range(NXC):
                            xts[c] = load_x_chunk(c)
                            qk_chunk(
                                m, c, xts[c], wqm, wkm, qt, kt,
                                projs=("q", "k") if c < 2 else ("k",),
                            )

                # ---- attention units (Q c2/c3 deferred after qg0) ----
                for qg in range(NQG):
                    q0 = qg * QG
                    for h in range(2):
                        head = 2 * m + h
                        p0 = h * 64
                        with band(att_cur):
                            if m == 0 and qg == 0 and h == 0:
                                emit_attention_unit(
                                    m, qg, h, q0, p0, head, qt, kt, qw=512
                                )
                                emit_attention_unit(
                                    m, qg, h, q0 + 512, p0, head, qt, kt, qw=512
                                )
                            else:
                                emit_attention_unit(
                                    m, qg, h, q0, p0, head, qt, kt,
                                    last=(m == NPAIR - 1 and qg == NQG - 1 and h == 1),
                                )
                    if qg == 0 and m > 0:
                        with band(fill_cur):
                            for c in (2, 3):
                                qk_chunk(m, c, xts[c], wqm, wkm, qt, kt, projs=("q",))


    nc.compile()
    return nc


def _get_nc():
    if "nc" not in _CACHE:
        _CACHE["nc"] = _build()
    return _CACHE["nc"]


def _in_maps(x, Wq, bq, Wk, bk, Wv, bv):
    maps = []
    for c in range(NCORES):
        b, hh = c // 2, c % 2
        cs = slice(hh * COLS, (hh + 1) * COLS)
        maps.append(
            {
                "xT": np.ascontiguousarray(np.asarray(x)[b].T),
                "wq": np.ascontiguousarray(np.asarray(Wq)[:, cs]),
                "wk": np.ascontiguousarray(np.asarray(Wk)[:, cs]),
                "wv": np.ascontiguousarray(np.asarray(Wv)[:, cs]),
                "bq": np.ascontiguousarray(np.asarray(bq)[cs]),
                "bk": np.ascontiguousarray(np.asarray(bk)[cs]),
                "bv": np.ascontiguousarray(np.asarray(bv)[cs]),
            }
        )
    return maps


def _run(inputs, trace=False):
    from concourse import bass_utils

    nc = _get_nc()
    res = bass_utils.run_bass_kernel_spmd(
        nc,
        _in_maps(**inputs),
        core_ids=list(range(NCORES)),
        trace=trace,
    )
    out = np.empty((B, S, D), np.float32)
    for c in range(NCORES):
        b, hh = c // 2, c % 2
        out[b, :, hh * COLS : (hh + 1) * COLS] = res.results[c]["out"].T
    return out, res


def kernel(**inputs):
    out, _ = _run(inputs, trace=False)
    return out


if __name__ == "__main__":
    _get_nc()
    print("build ok")



# revision 3
# speedup vs baseline: 1.0234x; 1.0234x over previous
"""Trainium2 Bass kernel for nn_AttentionLayer (B=4, S=2048, H=16, DH=64).

Sharding: 8 cores = 4 batches x 2 head-halves. Core c handles batch c//2,
heads (c%2)*8 .. (c%2)*8+8 (i.e. 512 of the 1024 QKV columns).

Per-core device program (SPMD, same program on all cores, different inputs):
  inputs (pre-laid-out on host, x/W in bf16 to halve DMA):
    xT  [1024, 2048] bf16 = x[b].T       (contraction dim on partitions)
    wq/wk/wv [1024, 512] bf16            (column slice for this core's heads)
    bq/bk/bv [512] f32
  output:
    out [2048, 512] f32 = attention ctx, seq-major (q rows, head*64+dh cols);
    host only reshapes.

v3 design (vs baseline, which ran PV as V_aug.T @ E at f32r N=512):
  - QKV projections from bf16 x/W (same modeled PE rate, half the HBM
    traffic; input DMA was starving the exp pipeline at startup).
  - Scores kept [128 kpos, q] f32r (N=512 matmuls, full rate); exp on ACT
    (PSUM->SBUF, bf16 out). ACT is the modeled bottleneck engine:
    256 ops x (1024*0.833 + 185)ns ~ 266us busy, at its floor.
  - PV transposed: ctx[q, dh] += E[:, q128].T @ V_aug[kb], with
    V_aug = [V | ones] bf16 [128, 65]; N=65 at bf16 full rate. Column 64
    accumulates the softmax denominator for free. This is the big PE win:
    PV drops from 512 to 65 cycles per matmul (f32r would be 4x-penalized
    at N<256); total PE ~721k -> ~592k cycles (~300 -> ~247us busy).
  - PSUM zero-region rule (start=True lazily zeroes the whole 2KB bank):
    each unit's ctx lives in TWO banks of [128, 4, 65] with exactly one
    accumulation group per bank (start at kb==0 s==first, stop at kb==15
    s==last). PSUM: scores 2x2 banks + ctx 2x1 + qkv 2 = 8.
  - Normalize straight from PSUM on DVE: reciprocal of the denominator
    column then per-subtile scalar_tensor_tensor (ctx * (1/den)) + bv
    (bv folded post-normalize since sum(p)=1), seq-major DMA out per
    4-subtile half.
"""

import numpy as np

B, S, H, DH = 4, 2048, 16, 64
D = H * DH  # 1024
NCORES = 8
COLS = 512  # qkv columns per core (8 heads)
NPAIR = 4  # head pairs per core
NKB = S // 128  # 16 k-blocks
QG = 1024  # q-group width
NQG = S // QG  # 2
XC = 512  # x streaming chunk (seq cols)
NXC = S // XC  # 4
INV_SQRT_DH = 1.0 / 8.0

_CACHE = {}


def _build():
    import concourse.mybir as mybir
    import concourse.tile as tile
    from concourse import bacc

    f32 = mybir.dt.float32
    f32r = mybir.dt.float32r
    bf16 = mybir.dt.bfloat16
    Exp = mybir.ActivationFunctionType.Exp
    Alu = mybir.AluOpType

    nc = bacc.Bacc(
        "TRN2",
        target_bir_lowering=False,
        debug=False,
        enable_asserts=False,
        num_devices=NCORES,
    )

    xT_d = nc.dram_tensor("xT", [D, S], bf16, kind="ExternalInput").ap()
    wq_d = nc.dram_tensor("wq", [D, COLS], bf16, kind="ExternalInput").ap()
    wk_d = nc.dram_tensor("wk", [D, COLS], bf16, kind="ExternalInput").ap()
    wv_d = nc.dram_tensor("wv", [D, COLS], bf16, kind="ExternalInput").ap()
    bq_d = nc.dram_tensor("bq", [COLS], f32, kind="ExternalInput").ap()
    bk_d = nc.dram_tensor("bk", [COLS], f32, kind="ExternalInput").ap()
    bv_d = nc.dram_tensor("bv", [COLS], f32, kind="ExternalInput").ap()
    out_d = nc.dram_tensor("out", [S, COLS], f32, kind="ExternalOutput").ap()

    with tile.TileContext(nc) as tc:
        with (
            tc.tile_pool(name="consts", bufs=1) as consts,
            tc.tile_pool(name="vpool", bufs=1) as vpool,
            tc.tile_pool(name="wvpool", bufs=1) as wvpool,
            tc.tile_pool(name="wqk", bufs=2) as wqk,
            tc.tile_pool(name="xpool", bufs=2) as xpool,
            tc.tile_pool(name="qkt", bufs=2) as qkt,
            tc.tile_pool(name="epool", bufs=8) as epool,
            tc.tile_pool(name="opool", bufs=4) as opool,
            tc.tile_pool(name="psum", bufs=1, space="PSUM") as psum,
        ):
            # ---- constants, ACT table preload ----
            bq_t = consts.tile([128, NPAIR], f32)
            bk_t = consts.tile([128, NPAIR], f32)
            bv_s = consts.tile([1, COLS], f32)
            bvb = consts.tile([128, 8, DH], f32)  # bv broadcast per head
            nc.gpsimd.dma_start(out=bq_t, in_=bq_d.rearrange("(m p) -> p m", p=128))
            nc.gpsimd.dma_start(out=bk_t, in_=bk_d.rearrange("(m p) -> p m", p=128))
            nc.gpsimd.dma_start(out=bv_s, in_=bv_d[None, :])
            for h in range(8):
                nc.gpsimd.partition_broadcast(
                    bvb[:, h, :], bv_s[:, h * DH : (h + 1) * DH]
                )
            warm = consts.tile([1, 1], f32)
            nc.vector.memset(warm, 0.0)
            nc.scalar.activation(warm, warm, Exp)  # pull ACT table load early

            # V_aug, bf16: vt[i][:, h, 0:64] = V chunk, col 64 = ones (denom)
            vt = [vpool.tile([128, 8, 65], bf16, name=f"vt{i}") for i in range(NKB)]
            for i in range(NKB):
                nc.vector.memset(vt[i][:, :, 64:65], 1.0)

            wv = wvpool.tile([128, 8, COLS], bf16, name="wv")

            def load_wv():
                nc.sync.dma_start(
                    out=wv, in_=wv_d.rearrange("(j p) c -> p j c", p=128)
                )

            def load_x_chunk(c, wpair=None, eng=None):
                if wpair is not None:
                    load_w_dma(*wpair)
                if eng is None:
                    eng = nc.sync
                xt = xpool.tile([128, 8, XC], bf16, name="xt", tag="xt")
                for j in range(8):
                    eng.dma_start(
                        out=xt[:, j, :],
                        in_=xT_d[j * 128 : (j + 1) * 128, c * XC : (c + 1) * XC],
                    )
                return xt

            def load_w_dma(wqm, wkm, m):
                nc.sync.dma_start(
                    out=wkm,
                    in_=wk_d[:, m * 128 : (m + 1) * 128].rearrange(
                        "(j p) c -> p j c", p=128
                    ),
                )
                nc.sync.dma_start(
                    out=wqm,
                    in_=wq_d[:, m * 128 : (m + 1) * 128].rearrange(
                        "(j p) c -> p j c", p=128
                    ),
                )

            def load_w_pair(m, defer_dma=False):
                wqm = wqk.tile([128, 8, 128], bf16, name="wqm", tag="wqm")
                wkm = wqk.tile([128, 8, 128], bf16, name="wkm", tag="wkm")
                if not defer_dma:
                    load_w_dma(wqm, wkm, m)
                return wqm, wkm

            def qk_chunk(m, c, xt, wqm, wkm, qt, kt, projs=("k", "q")):
                pairs = {"q": (wqm, bq_t, qt), "k": (wkm, bk_t, kt)}
                for w, bias, dst in (pairs[p] for p in projs):
                    ps = psum.tile([128, 512], f32, tag="p1", bufs=2)
                    for j in range(8):
                        nc.tensor.matmul(
                            ps,
                            lhsT=w[:, j, :],
                            rhs=xt[:, j, :],
                            start=(j == 0),
                            stop=(j == 7),
                        )
                    nc.vector.tensor_scalar_add(
                        dst[:, c * XC : (c + 1) * XC], ps, bias[:, m : m + 1]
                    )

            def v_chunk(c, xt):
                for i in range(XC // 128):
                    it = c * (XC // 128) + i
                    ps = psum.tile([128, 512], f32, tag="p1", bufs=2)
                    for j in range(8):
                        nc.tensor.matmul(
                            ps,
                            lhsT=xt[:, j, i * 128 : (i + 1) * 128],
                            rhs=wv[:, j, :],
                            start=(j == 0),
                            stop=(j == 7),
                        )
                    # bv folded in post-normalize (sum p = 1): plain cast copy
                    nc.vector.tensor_copy(
                        vt[it][:, :, 0:64], ps.rearrange("p (h d) -> p h d", h=8)
                    )

            def emit_attention_unit(m, q0, p0, head, qt, kt, qw=QG):
                nsub = qw // 128
                nhalf = (nsub + 3) // 4
                ctx = [
                    psum.tile(
                        [128, 4, 65], f32, tag=f"ctx{i}", bufs=1, name=f"ctx{i}"
                    )
                    for i in range(nhalf)
                ]

                def scores(kb):
                    sc = psum.tile([128, QG], f32, tag="sc", bufs=2, name="sc")
                    for qq in range(qw // 512):
                        nc.tensor.matmul(
                            sc[:, qq * 512 : (qq + 1) * 512],
                            lhsT=kt[p0 : p0 + 64, kb * 128 : (kb + 1) * 128],
                            rhs=qt[
                                p0 : p0 + 64,
                                q0 + qq * 512 : q0 + (qq + 1) * 512,
                            ],
                            start=True,
                            stop=True,
                        )
                    return sc

                # software-pipelined emission, two scores ahead (see baseline)
                scs = [scores(0), scores(1)]
                for kb in range(NKB):
                    ee = epool.tile([128, QG], bf16, tag="e", name="ee")
                    nc.scalar.activation(
                        ee[:, 0:qw], scs[kb % 2][:, 0:qw], Exp, scale=INV_SQRT_DH
                    )
                    if kb < NKB - 2:
                        scs[kb % 2] = scores(kb + 2)
                    for s in range(nsub):
                        # one accumulation group per PSUM bank (zero region):
                        # start on the bank's first write, stop on its last
                        nc.tensor.matmul(
                            ctx[s // 4][:, s % 4, :],
                            lhsT=ee[:, s * 128 : (s + 1) * 128],
                            rhs=vt[kb][:, head, :],
                            start=(kb == 0 and s % 4 == 0),
                            stop=(kb == NKB - 1 and s % 4 == 3),
                        )
                # normalize + bias straight from PSUM, seq-major DMA per half
                for i in range(nhalf):
                    rr = opool.tile([128, 4, 1], f32, tag="r", name="rr")
                    nc.vector.reciprocal(rr, ctx[i][:, :, 64:65])
                    ob = opool.tile([128, 4, DH], f32, tag="o", name="ob")
                    for s in range(4):
                        nc.vector.scalar_tensor_tensor(
                            ob[:, s, :],
                            ctx[i][:, s, 0:64],
                            rr[:, s, :],
                            bvb[:, head, :],
                            op0=Alu.mult,
                            op1=Alu.add,
                        )
                    nc.sync.dma_start(
                        out=out_d[
                            q0 + i * 512 : q0 + (i + 1) * 512,
                            head * DH : (head + 1) * DH,
                        ].rearrange("(s p) d -> p s d", p=128),
                        in_=ob,
                    )

            # ---- banded priorities: attention preferred, QKV/V fill gaps ----
            from contextlib import contextmanager

            base = tc.cur_priority + 50
            att_cur = [base]
            fill_cur = [base + 6000]

            @contextmanager
            def band(cursor):
                off = tc.cur_priority - cursor[0]
                with tc.high_priority(offset=off):
                    yield
                    cursor[0] = tc.cur_priority

            # ---- per pair: QKV (filler band) then attention (att band) ----
            for m in range(NPAIR):
                with band(fill_cur):
                    wqm, wkm = load_w_pair(m, defer_dma=(m == 0))
                    qt = qkt.tile([128, S], f32r, name=f"qt{m}", tag="qt")
                    kt = qkt.tile([128, S], f32r, name=f"kt{m}", tag="kt")
                    xts = {}
                    if m == 0:
                        xts[0] = load_x_chunk(0, wpair=(wqm, wkm, m))
                        xts[1] = load_x_chunk(1)
                        qk_chunk(m, 0, xts[0], wqm, wkm, qt, kt)
                        qk_chunk(m, 1, xts[1], wqm, wkm, qt, kt)
                        load_wv()
                        v_chunk(0, xts[0])
                        v_chunk(1, xts[1])
                        for c in (2, 3):
                            xts[c] = load_x_chunk(c)
                            qk_chunk(m, c, xts[c], wqm, wkm, qt, kt)
                            v_chunk(c, xts[c])
                    else:
                        for c in range(NXC):
                            xts[c] = load_x_chunk(c)
                            qk_chunk(
                                m, c, xts[c], wqm, wkm, qt, kt,
                                projs=("q", "k") if c < 2 else ("k",),
                            )

                # ---- attention units (Q c2/c3 deferred after qg0) ----
                for qg in range(NQG):
                    q0 = qg * QG
                    for h in range(2):
                        head = 2 * m + h
                        p0 = h * 64
                        with band(att_cur):
                            if m == 0 and qg == 0 and h == 0:
                                emit_attention_unit(
                                    m, q0, p0, head, qt, kt, qw=512
                                )
                                emit_attention_unit(
                                    m, q0 + 512, p0, head, qt, kt, qw=512
                                )
                            else:
                                emit_attention_unit(m, q0, p0, head, qt, kt)
                    if qg == 0 and m > 0:
                        with band(fill_cur):
                            for c in (2, 3):
                                qk_chunk(m, c, xts[c], wqm, wkm, qt, kt, projs=("q",))

    nc.compile()
    return nc


def _get_nc():
    if "nc" not in _CACHE:
        _CACHE["nc"] = _build()
    return _CACHE["nc"]


def _in_maps(x, Wq, bq, Wk, bk, Wv, bv):
    import ml_dtypes

    bf = ml_dtypes.bfloat16
    maps = []
    for c in range(NCORES):
        b, hh = c // 2, c % 2
        cs = slice(hh * COLS, (hh + 1) * COLS)
        maps.append(
            {
                "xT": np.ascontiguousarray(np.asarray(x)[b].T).astype(bf),
                "wq": np.ascontiguousarray(np.asarray(Wq)[:, cs]).astype(bf),
                "wk": np.ascontiguousarray(np.asarray(Wk)[:, cs]).astype(bf),
                "wv": np.ascontiguousarray(np.asarray(Wv)[:, cs]).astype(bf),
                "bq": np.ascontiguousarray(np.asarray(bq)[cs]),
                "bk": np.ascontiguousarray(np.asarray(bk)[cs]),
                "bv": np.ascontiguousarray(np.asarray(bv)[cs]),
            }
        )
    return maps


def _run(inputs, trace=False):
    from concourse import bass_utils

    nc = _get_nc()
    res = bass_utils.run_bass_kernel_spmd(
        nc,
        _in_maps(**inputs),
        core_ids=list(range(NCORES)),
        trace=trace,
    )
    out = np.empty((B, S, D), np.float32)
    for c in range(NCORES):
        b, hh = c // 2, c % 2
        out[b, :, hh * COLS : (hh + 1) * COLS] = res.results[c]["out"]
    return out, res


def kernel(**inputs):
    out, _ = _run(inputs, trace=False)
    return out


if __name__ == "__main__":
    _get_nc()
    print("build ok")


# revision 20
# speedup vs baseline: 1.0845x; 1.0597x over previous
"""Trainium2 Bass kernel for nn_AttentionLayer (B=4, S=2048, H=16, DH=64).

Sharding: 8 cores = 4 batches x 2 head-halves. Core c handles batch c//2,
heads (c%2)*8 .. (c%2)*8+8 (i.e. 512 of the 1024 QKV columns).

Per-core device program (SPMD, same program on all cores, different inputs):
  inputs (pre-laid-out on host, x/W in bf16 to halve DMA):
    xT  [1024, 2048] bf16 = x[b].T       (contraction dim on partitions)
    wq/wk/wv [1024, 512] bf16            (column slice for this core's heads)
    bq/bk/bv [512] f32
  output:
    out [2048, 512] f32 = attention ctx, seq-major (q rows, head*64+dh cols);
    host only reshapes.

v3 design (vs baseline, which ran PV as V_aug.T @ E at f32r N=512):
  - QKV projections from bf16 x/W (same modeled PE rate, half the HBM
    traffic; input DMA was starving the exp pipeline at startup).
  - Scores kept [128 kpos, q] f32r (N=512 matmuls, full rate); exp on ACT
    (PSUM->SBUF, bf16 out). ACT is the modeled bottleneck engine:
    256 ops x (1024*0.833 + 185)ns ~ 266us busy, at its floor.
  - PV transposed: ctx[q, dh] += E[:, q128].T @ V_aug[kb], with
    V_aug = [V | ones] bf16 [128, 65]; N=65 at bf16 full rate. Column 64
    accumulates the softmax denominator for free. This is the big PE win:
    PV drops from 512 to 65 cycles per matmul (f32r would be 4x-penalized
    at N<256); total PE ~721k -> ~592k cycles (~300 -> ~247us busy).
  - PSUM zero-region rule (start=True lazily zeroes the whole 2KB bank):
    each unit's ctx lives in TWO banks of [128, 4, 65] with exactly one
    accumulation group per bank (start at kb==0 s==first, stop at kb==15
    s==last). PSUM: scores 2x2 banks + ctx 2x1 + qkv 2 = 8.
  - Normalize straight from PSUM on DVE: reciprocal of the denominator
    column then per-subtile scalar_tensor_tensor (ctx * (1/den)) + bv
    (bv folded post-normalize since sum(p)=1), seq-major DMA out per
    4-subtile half.
"""

import numpy as np

B, S, H, DH = 4, 2048, 16, 64
D = H * DH  # 1024
NCORES = 8
COLS = 512  # qkv columns per core (8 heads)
NPAIR = 4  # head pairs per core
NKB = S // 128  # 16 k-blocks
QG = 1024  # q-group width
NQG = S // QG  # 2
XC = 512  # x streaming chunk (seq cols)
NXC = S // XC  # 4
INV_SQRT_DH = 1.0 / 8.0

_CACHE = {}


def _build():
    import concourse.mybir as mybir
    import concourse.tile as tile
    from concourse import bacc

    f32 = mybir.dt.float32
    f32r = mybir.dt.float32r
    bf16 = mybir.dt.bfloat16
    Exp = mybir.ActivationFunctionType.Exp
    Alu = mybir.AluOpType

    nc = bacc.Bacc(
        "TRN2",
        target_bir_lowering=False,
        debug=False,
        enable_asserts=False,
        num_devices=NCORES,
    )

    xT_d = nc.dram_tensor("xT", [D, S], bf16, kind="ExternalInput").ap()
    # Wk/Wq host-pre-rearranged to the SBUF tile layout [128, 8, 128]:
    # 2KB contiguous rows -> full descriptor rate, 728ns per pair load
    wk2_d = nc.dram_tensor("wk2", [NPAIR, 128, 8, 128], bf16, kind="ExternalInput").ap()
    wq2_d = nc.dram_tensor("wq2", [NPAIR, 128, 8, 128], bf16, kind="ExternalInput").ap()
    wv_d = nc.dram_tensor("wv", [D, COLS], bf16, kind="ExternalInput").ap()
    bq_d = nc.dram_tensor("bq", [COLS], f32, kind="ExternalInput").ap()
    bk_d = nc.dram_tensor("bk", [COLS], f32, kind="ExternalInput").ap()
    bv_d = nc.dram_tensor("bv", [COLS], f32, kind="ExternalInput").ap()
    out_d = nc.dram_tensor("out", [S, COLS], f32, kind="ExternalOutput").ap()

    with tile.TileContext(nc) as tc:
        with (
            tc.tile_pool(name="consts", bufs=1) as consts,
            tc.tile_pool(name="vpool", bufs=1) as vpool,
            tc.tile_pool(name="wvpool", bufs=1) as wvpool,
            tc.tile_pool(name="wqk", bufs=2) as wqk,
            tc.tile_pool(name="xpool", bufs=1) as xpool,
            tc.tile_pool(name="qkt", bufs=2) as qkt,
            tc.tile_pool(name="epool", bufs=44) as epool,
            tc.tile_pool(name="opool", bufs=4) as opool,
            tc.tile_pool(name="psum", bufs=1, space="PSUM") as psum,
        ):
            # ---- constants, ACT table preload ----
            bq_t = consts.tile([128, NPAIR], f32)
            bk_t = consts.tile([128, NPAIR], f32)
            bv_s = consts.tile([1, COLS], f32)
            bvb = consts.tile([128, 8, DH], f32)  # bv broadcast per head
            nc.gpsimd.dma_start(out=bq_t, in_=bq_d.rearrange("(m p) -> p m", p=128))
            nc.gpsimd.dma_start(out=bk_t, in_=bk_d.rearrange("(m p) -> p m", p=128))
            nc.gpsimd.dma_start(out=bv_s, in_=bv_d[None, :])
            for h in range(8):
                nc.gpsimd.partition_broadcast(
                    bvb[:, h, :], bv_s[:, h * DH : (h + 1) * DH]
                )
            warm = consts.tile([1, 1], f32)
            nc.vector.memset(warm, 0.0)
            nc.scalar.activation(warm, warm, Exp)  # pull ACT table load early
            # ramp the PE p-state during the initial DMA wait: ~4us of dummy
            # matmuls so the first real projection runs at full clock
            wzero = consts.tile([128, 512], bf16)
            nc.vector.memset(wzero, 0.0)
            wps = psum.tile([128, 512], f32, tag="p1", bufs=2, name="wps")
            for i in range(8):
                nc.tensor.matmul(
                    wps,
                    lhsT=wzero[:, 0:128],
                    rhs=wzero,
                    start=(i == 0),
                    stop=(i == 7),
                )

            # V_aug, bf16: vt[i][:, h, 0:64] = V chunk, col 64 = ones (denom)
            vt = [vpool.tile([128, 8, 65], bf16, name=f"vt{i}") for i in range(NKB)]
            for i in range(NKB):
                nc.vector.memset(vt[i][:, :, 64:65], 1.0)

            wv = wvpool.tile([128, 8, COLS], bf16, name="wv")

            def load_wv():
                nc.sync.dma_start(
                    out=wv, in_=wv_d.rearrange("(j p) c -> p j c", p=128)
                )

            def load_x_chunk(c):
                # x[b] in bf16 is only 32KB/partition: stream it ONCE and keep
                # it resident; every head-pair's projections reuse it
                xt = xpool.tile([128, 8, XC], bf16, name=f"xt{c}", tag=f"xt{c}")
                # two half-issues: projection chains start on the first half
                # (8 per-j dma_starts would serialize on HWDGE issue overhead)
                for hh in range(2):
                    nc.sync.dma_start(
                        out=xt[:, hh * 4 : (hh + 1) * 4, :],
                        in_=xT_d[
                            hh * 512 : (hh + 1) * 512, c * XC : (c + 1) * XC
                        ].rearrange("(j p) c -> p j c", p=128),
                    )
                return xt

            def load_w_pair(m, defer_dma=False):
                wkm = wqk.tile([128, 8, 128], bf16, name="wkm", tag="wkm")
                wqm = wqk.tile([128, 8, 128], bf16, name="wqm", tag="wqm")
                if not defer_dma:
                    nc.sync.dma_start(out=wkm, in_=wk2_d[m])
                    nc.sync.dma_start(out=wqm, in_=wq2_d[m])
                return wqm, wkm

            def qk_chunk(m, c, xt, wqm, wkm, qt, kt, projs=("k", "q")):
                pairs = {"q": (wqm, bq_t, qt), "k": (wkm, bk_t, kt)}
                for w, bias, dst in (pairs[p] for p in projs):
                    ps = psum.tile([128, 512], f32, tag="p1", bufs=2)
                    for j in range(8):
                        nc.tensor.matmul(
                            ps,
                            lhsT=w[:, j, :],
                            rhs=xt[:, j, :],
                            start=(j == 0),
                            stop=(j == 7),
                        )
                    nc.vector.tensor_scalar_add(
                        dst[:, c * XC : (c + 1) * XC], ps, bias[:, m : m + 1]
                    )

            def v_chunk(c, xt):
                for i in range(XC // 128):
                    it = c * (XC // 128) + i
                    ps = psum.tile([128, 512], f32, tag="p1", bufs=2)
                    for j in range(8):
                        nc.tensor.matmul(
                            ps,
                            lhsT=xt[:, j, i * 128 : (i + 1) * 128],
                            rhs=wv[:, j, :],
                            start=(j == 0),
                            stop=(j == 7),
                        )
                    # bv folded in post-normalize (sum p = 1): plain cast
                    # copy (DVE: GPSIMD cannot access PSUM on TRN2)
                    nc.vector.tensor_copy(
                        vt[it][:, :, 0:64], ps.rearrange("p (h d) -> p h d", h=8)
                    )

            def emit_attention_unit(m, q0, p0, head, qt, kt, qw=QG):
                nsub = qw // 128
                nhalf = (nsub + 3) // 4
                ctx = [
                    psum.tile(
                        [128, 4, 65], f32, tag=f"ctx{i}", bufs=1, name=f"ctx{i}"
                    )
                    for i in range(nhalf)
                ]

                def scores(kb):
                    sc = psum.tile([128, QG], f32, tag="sc", bufs=2, name="sc")
                    for qq in range(qw // 512):
                        nc.tensor.matmul(
                            sc[:, qq * 512 : (qq + 1) * 512],
                            lhsT=kt[p0 : p0 + 64, kb * 128 : (kb + 1) * 128],
                            rhs=qt[
                                p0 : p0 + 64,
                                q0 + qq * 512 : q0 + (qq + 1) * 512,
                            ],
                            start=True,
                            stop=True,
                        )
                    return sc

                # software-pipelined emission, two scores ahead (see baseline)
                scs = [scores(0), scores(1)]
                for kb in range(NKB):
                    ee = epool.tile([128, QG], bf16, tag="e", name="ee")
                    nc.scalar.activation(
                        ee[:, 0:qw], scs[kb % 2][:, 0:qw], Exp, scale=INV_SQRT_DH
                    )
                    if kb < NKB - 2:
                        scs[kb % 2] = scores(kb + 2)
                    for s in range(nsub):
                        # one accumulation group per PSUM bank (zero region):
                        # start on the bank's first write, stop on its last
                        nc.tensor.matmul(
                            ctx[s // 4][:, s % 4, :],
                            lhsT=ee[:, s * 128 : (s + 1) * 128],
                            rhs=vt[kb][:, head, :],
                            start=(kb == 0 and s % 4 == 0),
                            stop=(kb == NKB - 1 and s % 4 == 3),
                        )
                # normalize: one bulk copy PSUM->SBUF (frees the ctx bank for
                # the next unit's PV asap), then recip + per-subtile
                # (ctx * 1/den) + bv from SBUF, seq-major DMA per half
                for i in range(nhalf):
                    cs = opool.tile([128, 4, 65], f32, tag="c", name="cs")
                    nc.vector.tensor_copy(cs, ctx[i])
                    rr = opool.tile([128, 4, 1], f32, tag="r", name="rr")
                    nc.vector.reciprocal(rr, cs[:, :, 64:65])
                    ob = opool.tile([128, 4, DH], f32, tag="o", name="ob")
                    for s in range(4):
                        nc.vector.scalar_tensor_tensor(
                            ob[:, s, :],
                            cs[:, s, 0:64],
                            rr[:, s, :],
                            bvb[:, head, :],
                            op0=Alu.mult,
                            op1=Alu.add,
                        )
                    nc.sync.dma_start(
                        out=out_d[
                            q0 + i * 512 : q0 + (i + 1) * 512,
                            head * DH : (head + 1) * DH,
                        ].rearrange("(s p) d -> p s d", p=128),
                        in_=ob,
                    )

            # ---- banded priorities: attention preferred, QKV/V fill gaps ----
            from contextlib import contextmanager

            base = tc.cur_priority + 50
            att_cur = [base]
            fill_cur = [base + 6000]

            @contextmanager
            def band(cursor):
                off = tc.cur_priority - cursor[0]
                with tc.high_priority(offset=off):
                    yield
                    cursor[0] = tc.cur_priority

            # ---- per pair: QKV (filler band) then attention (att band) ----
            xts = {}
            for m in range(NPAIR):
                with band(fill_cur):
                    wqm, wkm = load_w_pair(m, defer_dma=(m == 0))
                    qt = qkt.tile([128, S], bf16, name=f"qt{m}", tag="qt")
                    kt = qkt.tile([128, S], bf16, name=f"kt{m}", tag="kt")
                    if m == 0:
                        # all QK chunks first (kt feeds every unit's scores;
                        # exp stalls if kt c2/c3 are late), V strictly after
                        nc.sync.dma_start(out=wkm, in_=wk2_d[m])
                        xts[0] = load_x_chunk(0)
                        nc.sync.dma_start(out=wqm, in_=wq2_d[m])
                        xts[1] = load_x_chunk(1)
                        load_wv()
                        xts[2] = load_x_chunk(2)
                        xts[3] = load_x_chunk(3)
                        qk_chunk(m, 0, xts[0], wqm, wkm, qt, kt)
                        qk_chunk(m, 1, xts[1], wqm, wkm, qt, kt)
                        qk_chunk(m, 2, xts[2], wqm, wkm, qt, kt)
                        qk_chunk(m, 3, xts[3], wqm, wkm, qt, kt)
                        for c in range(NXC):
                            v_chunk(c, xts[c])
                    else:
                        for c in range(NXC):
                            qk_chunk(
                                m, c, xts[c], wqm, wkm, qt, kt,
                                projs=("k", "q") if c < 2 else ("k",),
                            )

                # ---- attention units (Q c2/c3 deferred after qg0) ----
                for qg in range(NQG):
                    q0 = qg * QG
                    for h in range(2):
                        head = 2 * m + h
                        p0 = h * 64
                        with band(att_cur):
                            emit_attention_unit(m, q0, p0, head, qt, kt)
                    if qg == 0 and m > 0:
                        with band(fill_cur):
                            for c in (2, 3):
                                qk_chunk(m, c, xts[c], wqm, wkm, qt, kt, projs=("q",))

    nc.compile()
    return nc


def _get_nc():
    if "nc" not in _CACHE:
        _CACHE["nc"] = _build()
    return _CACHE["nc"]


def _w_slab(w):
    # [D, 512] -> [NPAIR, 128, 8, 128]: per pair m take cols m*128:(m+1)*128,
    # rows (j*128+p) -> [p, j, c] (the SBUF tile layout, contiguous rows)
    import ml_dtypes

    out = np.empty((NPAIR, 128, 8, 128), ml_dtypes.bfloat16)
    for m in range(NPAIR):
        out[m] = (
            w[:, m * 128 : (m + 1) * 128].reshape(8, 128, 128).transpose(1, 0, 2)
        ).astype(ml_dtypes.bfloat16)
    return np.ascontiguousarray(out)


def _in_maps(x, Wq, bq, Wk, bk, Wv, bv):
    import ml_dtypes

    bf = ml_dtypes.bfloat16
    maps = []
    for c in range(NCORES):
        b, hh = c // 2, c % 2
        cs = slice(hh * COLS, (hh + 1) * COLS)
        maps.append(
            {
                "xT": np.ascontiguousarray(np.asarray(x)[b].T).astype(bf),
                "wk2": _w_slab(np.asarray(Wk)[:, cs]),
                "wq2": _w_slab(np.asarray(Wq)[:, cs]),
                "wv": np.ascontiguousarray(np.asarray(Wv)[:, cs]).astype(bf),
                "bq": np.ascontiguousarray(np.asarray(bq)[cs]),
                "bk": np.ascontiguousarray(np.asarray(bk)[cs]),
                "bv": np.ascontiguousarray(np.asarray(bv)[cs]),
            }
        )
    return maps


def _run(inputs, trace=False):
    from concourse import bass_utils

    nc = _get_nc()
    res = bass_utils.run_bass_kernel_spmd(
        nc,
        _in_maps(**inputs),
        core_ids=list(range(NCORES)),
        trace=trace,
    )
    out = np.empty((B, S, D), np.float32)
    for c in range(NCORES):
        b, hh = c // 2, c % 2
        out[b, :, hh * COLS : (hh + 1) * COLS] = res.results[c]["out"]
    return out, res


def kernel(**inputs):
    out, _ = _run(inputs, trace=False)
    return out


if __name__ == "__main__":
    _get_nc()
    print("build ok")


# revision 24
# speedup vs baseline: 1.0858x; 1.0012x over previous
"""Trainium2 Bass kernel for nn_AttentionLayer (B=4, S=2048, H=16, DH=64).

Sharding: 8 cores = 4 batches x 2 head-halves. Core c handles batch c//2,
heads (c%2)*8 .. (c%2)*8+8 (i.e. 512 of the 1024 QKV columns).

Per-core device program (SPMD, same program on all cores, different inputs):
  inputs (pre-laid-out on host, x/W in bf16 to halve DMA):
    xT  [1024, 2048] bf16 = x[b].T       (contraction dim on partitions)
    wq/wk/wv [1024, 512] bf16            (column slice for this core's heads)
    bq/bk/bv [512] f32
  output:
    out [2048, 512] f32 = attention ctx, seq-major (q rows, head*64+dh cols);
    host only reshapes.

v3 design (vs baseline, which ran PV as V_aug.T @ E at f32r N=512):
  - QKV projections from bf16 x/W (same modeled PE rate, half the HBM
    traffic; input DMA was starving the exp pipeline at startup).
  - Scores kept [128 kpos, q] f32r (N=512 matmuls, full rate); exp on ACT
    (PSUM->SBUF, bf16 out). ACT is the modeled bottleneck engine:
    256 ops x (1024*0.833 + 185)ns ~ 266us busy, at its floor.
  - PV transposed: ctx[q, dh] += E[:, q128].T @ V_aug[kb], with
    V_aug = [V | ones] bf16 [128, 65]; N=65 at bf16 full rate. Column 64
    accumulates the softmax denominator for free. This is the big PE win:
    PV drops from 512 to 65 cycles per matmul (f32r would be 4x-penalized
    at N<256); total PE ~721k -> ~592k cycles (~300 -> ~247us busy).
  - PSUM zero-region rule (start=True lazily zeroes the whole 2KB bank):
    each unit's ctx lives in TWO banks of [128, 4, 65] with exactly one
    accumulation group per bank (start at kb==0 s==first, stop at kb==15
    s==last). PSUM: scores 2x2 banks + ctx 2x1 + qkv 2 = 8.
  - Normalize straight from PSUM on DVE: reciprocal of the denominator
    column then per-subtile scalar_tensor_tensor (ctx * (1/den)) + bv
    (bv folded post-normalize since sum(p)=1), seq-major DMA out per
    4-subtile half.
"""

import numpy as np

B, S, H, DH = 4, 2048, 16, 64
D = H * DH  # 1024
NCORES = 8
COLS = 512  # qkv columns per core (8 heads)
NPAIR = 4  # head pairs per core
NKB = S // 128  # 16 k-blocks
QG = 1024  # q-group width
NQG = S // QG  # 2
XC = 512  # x streaming chunk (seq cols)
NXC = S // XC  # 4
INV_SQRT_DH = 1.0 / 8.0

_CACHE = {}


def _build():
    import concourse.mybir as mybir
    import concourse.tile as tile
    from concourse import bacc

    f32 = mybir.dt.float32
    f32r = mybir.dt.float32r
    bf16 = mybir.dt.bfloat16
    Exp = mybir.ActivationFunctionType.Exp
    Alu = mybir.AluOpType

    nc = bacc.Bacc(
        "TRN2",
        target_bir_lowering=False,
        debug=False,
        enable_asserts=False,
        num_devices=NCORES,
    )

    xT_d = nc.dram_tensor("xT", [D, S], bf16, kind="ExternalInput").ap()
    # Wk|Wq host-pre-rearranged+packed to SBUF tile layout [128, 8, 256]
    # (wk in cols 0:128, wq in 128:256): 4KB contiguous rows, one full-rate
    # dma_start per pair for both weights
    wkq_d = nc.dram_tensor(
        "wkq", [NPAIR, 128, 8, 256], bf16, kind="ExternalInput"
    ).ap()
    wv_d = nc.dram_tensor("wv", [D, COLS], bf16, kind="ExternalInput").ap()
    bq_d = nc.dram_tensor("bq", [COLS], f32, kind="ExternalInput").ap()
    bk_d = nc.dram_tensor("bk", [COLS], f32, kind="ExternalInput").ap()
    bv_d = nc.dram_tensor("bv", [COLS], f32, kind="ExternalInput").ap()
    out_d = nc.dram_tensor("out", [S, COLS], bf16, kind="ExternalOutput").ap()

    with tile.TileContext(nc) as tc:
        with (
            tc.tile_pool(name="consts", bufs=1) as consts,
            tc.tile_pool(name="vpool", bufs=1) as vpool,
            tc.tile_pool(name="wvpool", bufs=1) as wvpool,
            tc.tile_pool(name="wqk", bufs=2) as wqk,
            tc.tile_pool(name="xpool", bufs=1) as xpool,
            tc.tile_pool(name="qkt", bufs=2) as qkt,
            tc.tile_pool(name="epool", bufs=48) as epool,
            tc.tile_pool(name="opool", bufs=3) as opool,
            tc.tile_pool(name="psum", bufs=1, space="PSUM") as psum,
        ):
            # ---- constants, ACT table preload ----
            bq_t = consts.tile([128, NPAIR], f32)
            bk_t = consts.tile([128, NPAIR], f32)
            bv_s = consts.tile([1, COLS], f32)
            bvb = consts.tile([128, 8, DH], f32)  # bv broadcast per head
            nc.gpsimd.dma_start(out=bq_t, in_=bq_d.rearrange("(m p) -> p m", p=128))
            nc.gpsimd.dma_start(out=bk_t, in_=bk_d.rearrange("(m p) -> p m", p=128))
            nc.gpsimd.dma_start(out=bv_s, in_=bv_d[None, :])
            for h in range(8):
                nc.gpsimd.partition_broadcast(
                    bvb[:, h, :], bv_s[:, h * DH : (h + 1) * DH]
                )
            warm = consts.tile([1, 1], f32)
            nc.vector.memset(warm, 0.0)
            nc.scalar.activation(warm, warm, Exp)  # pull ACT table load early
            # ramp the PE p-state during the initial DMA wait: ~4us of dummy
            # matmuls so the first real projection runs at full clock
            wzero = consts.tile([128, 512], bf16)
            nc.vector.memset(wzero, 0.0)
            wps = psum.tile([128, 512], f32, tag="p1", bufs=2, name="wps")
            for i in range(8):
                nc.tensor.matmul(
                    wps,
                    lhsT=wzero[:, 0:128],
                    rhs=wzero,
                    start=(i == 0),
                    stop=(i == 7),
                )

            # V_aug, bf16: vt[i][:, h, 0:64] = V chunk, col 64 = ones (denom)
            vt = [vpool.tile([128, 8, 65], bf16, name=f"vt{i}") for i in range(NKB)]
            for i in range(NKB):
                nc.vector.memset(vt[i][:, :, 64:65], 1.0)

            wv = wvpool.tile([128, 8, COLS], bf16, name="wv")

            def load_wv():
                nc.sync.dma_start(
                    out=wv, in_=wv_d.rearrange("(j p) c -> p j c", p=128)
                )

            def load_x_chunk(c, fine=False):
                # x[b] in bf16 is only 32KB/partition: stream it ONCE and keep
                # it resident; every head-pair's projections reuse it.
                # 2 half-issues (4 quarter-issues for the startup-critical
                # chunks) so projection chains start on partial data; 8 per-j
                # dma_starts would serialize on HWDGE issue overhead.
                xt = xpool.tile([128, 8, XC], bf16, name=f"xt{c}", tag=f"xt{c}")
                n = 4 if fine else 2
                for hh in range(n):
                    nc.sync.dma_start(
                        out=xt[:, hh * 8 // n : (hh + 1) * 8 // n, :],
                        in_=xT_d[
                            hh * 1024 // n : (hh + 1) * 1024 // n,
                            c * XC : (c + 1) * XC,
                        ].rearrange("(j p) c -> p j c", p=128),
                    )
                return xt

            def load_w_pair(m, defer_dma=False):
                wkqm = wqk.tile([128, 8, 256], bf16, name="wkqm", tag="wkqm")
                if not defer_dma:
                    nc.sync.dma_start(out=wkqm, in_=wkq_d[m])
                return wkqm[:, :, 128:256], wkqm[:, :, 0:128], wkqm

            def qk_chunk(m, c, xt, wqm, wkm, qt, kt, projs=("k", "q")):
                pairs = {"q": (wqm, bq_t, qt), "k": (wkm, bk_t, kt)}
                for w, bias, dst in (pairs[p] for p in projs):
                    ps = psum.tile([128, 512], f32, tag="p1", bufs=2)
                    for j in range(8):
                        nc.tensor.matmul(
                            ps,
                            lhsT=w[:, j, :],
                            rhs=xt[:, j, :],
                            start=(j == 0),
                            stop=(j == 7),
                        )
                    nc.vector.tensor_scalar_add(
                        dst[:, c * XC : (c + 1) * XC], ps, bias[:, m : m + 1]
                    )

            def v_chunk(c, xt):
                for i in range(XC // 128):
                    it = c * (XC // 128) + i
                    ps = psum.tile([128, 512], f32, tag="p1", bufs=2)
                    for j in range(8):
                        nc.tensor.matmul(
                            ps,
                            lhsT=xt[:, j, i * 128 : (i + 1) * 128],
                            rhs=wv[:, j, :],
                            start=(j == 0),
                            stop=(j == 7),
                        )
                    # bv folded in post-normalize (sum p = 1): plain cast
                    # copy (DVE: GPSIMD cannot access PSUM on TRN2)
                    nc.vector.tensor_copy(
                        vt[it][:, :, 0:64], ps.rearrange("p (h d) -> p h d", h=8)
                    )

            def emit_attention_unit(m, q0, p0, head, qt, kt, qw=QG):
                nsub = qw // 128
                nhalf = (nsub + 3) // 4
                ctx = [
                    psum.tile(
                        [128, 4, 65], f32, tag=f"ctx{i}", bufs=1, name=f"ctx{i}"
                    )
                    for i in range(nhalf)
                ]

                def scores(kb):
                    sc = psum.tile([128, QG], f32, tag="sc", bufs=2, name="sc")
                    for qq in range(qw // 512):
                        nc.tensor.matmul(
                            sc[:, qq * 512 : (qq + 1) * 512],
                            lhsT=kt[p0 : p0 + 64, kb * 128 : (kb + 1) * 128],
                            rhs=qt[
                                p0 : p0 + 64,
                                q0 + qq * 512 : q0 + (qq + 1) * 512,
                            ],
                            start=True,
                            stop=True,
                        )
                    return sc

                # software-pipelined emission, two scores ahead (see baseline)
                scs = [scores(0), scores(1)]
                for kb in range(NKB):
                    ee = epool.tile([128, QG], bf16, tag="e", name="ee")
                    nc.scalar.activation(
                        ee[:, 0:qw], scs[kb % 2][:, 0:qw], Exp, scale=INV_SQRT_DH
                    )
                    if kb < NKB - 2:
                        scs[kb % 2] = scores(kb + 2)
                    for s in range(nsub):
                        # one accumulation group per PSUM bank (zero region):
                        # start on the bank's first write, stop on its last
                        nc.tensor.matmul(
                            ctx[s // 4][:, s % 4, :],
                            lhsT=ee[:, s * 128 : (s + 1) * 128],
                            rhs=vt[kb][:, head, :],
                            start=(kb == 0 and s % 4 == 0),
                            stop=(kb == NKB - 1 and s % 4 == 3),
                        )
                # normalize: bulk-copy BOTH ctx banks PSUM->SBUF first (frees
                # them for the next unit's PV asap), then recip + per-subtile
                # (ctx * 1/den) + bv from SBUF, seq-major DMA per half
                css = []
                for i in range(nhalf):
                    cs = opool.tile([128, 4, 65], f32, tag="c", name="cs")
                    nc.vector.tensor_copy(cs, ctx[i])
                    css.append(cs)
                for i, cs in enumerate(css):
                    rr = opool.tile([128, 4, 1], f32, tag="r", name="rr")
                    nc.vector.reciprocal(rr, cs[:, :, 64:65])
                    ob = opool.tile([128, 4, DH], bf16, tag="o", name="ob")
                    for s in range(4):
                        nc.vector.scalar_tensor_tensor(
                            ob[:, s, :],
                            cs[:, s, 0:64],
                            rr[:, s, :],
                            bvb[:, head, :],
                            op0=Alu.mult,
                            op1=Alu.add,
                        )
                    nc.sync.dma_start(
                        out=out_d[
                            q0 + i * 512 : q0 + (i + 1) * 512,
                            head * DH : (head + 1) * DH,
                        ].rearrange("(s p) d -> p s d", p=128),
                        in_=ob,
                    )

            # ---- banded priorities: attention preferred, QKV/V fill gaps ----
            from contextlib import contextmanager

            base = tc.cur_priority + 50
            att_cur = [base]
            fill_cur = [base + 6000]

            @contextmanager
            def band(cursor):
                off = tc.cur_priority - cursor[0]
                with tc.high_priority(offset=off):
                    yield
                    cursor[0] = tc.cur_priority

            # ---- per pair: QKV (filler band) then attention (att band) ----
            xts = {}
            for m in range(NPAIR):
                with band(fill_cur):
                    wqm, wkm, wkqm = load_w_pair(m, defer_dma=(m == 0))
                    qt = qkt.tile([128, S], bf16, name=f"qt{m}", tag="qt")
                    kt = qkt.tile([128, S], bf16, name=f"kt{m}", tag="kt")
                    if m == 0:
                        # all QK chunks first (kt feeds every unit's scores;
                        # exp stalls if kt c2/c3 are late), V strictly after
                        nc.sync.dma_start(out=wkqm, in_=wkq_d[m])
                        xts[0] = load_x_chunk(0, fine=True)
                        xts[1] = load_x_chunk(1, fine=True)
                        load_wv()
                        xts[2] = load_x_chunk(2)
                        xts[3] = load_x_chunk(3)
                        qk_chunk(m, 0, xts[0], wqm, wkm, qt, kt)
                        qk_chunk(m, 1, xts[1], wqm, wkm, qt, kt)
                        qk_chunk(m, 2, xts[2], wqm, wkm, qt, kt)
                        qk_chunk(m, 3, xts[3], wqm, wkm, qt, kt)
                        for c in range(NXC):
                            v_chunk(c, xts[c])
                    else:
                        for c in range(NXC):
                            qk_chunk(
                                m, c, xts[c], wqm, wkm, qt, kt,
                                projs=("k", "q") if c < 2 else ("k",),
                            )

                # ---- attention units (Q c2/c3 deferred after qg0) ----
                for qg in range(NQG):
                    q0 = qg * QG
                    for h in range(2):
                        head = 2 * m + h
                        p0 = h * 64
                        with band(att_cur):
                            emit_attention_unit(m, q0, p0, head, qt, kt)
                    if qg == 0 and m > 0:
                        with band(fill_cur):
                            for c in (2, 3):
                                qk_chunk(m, c, xts[c], wqm, wkm, qt, kt, projs=("q",))

    nc.compile()
    return nc


def _get_nc():
    if "nc" not in _CACHE:
        _CACHE["nc"] = _build()
    return _CACHE["nc"]


def _w_slab(wk, wq):
    # [D, 512] x2 -> [NPAIR, 128, 8, 256]: per pair m, wk cols m*128:(m+1)*128
    # at [..., 0:128] and wq's at [..., 128:256], rows (j*128+p) -> [p, j, c]
    # (the SBUF tile layout, 4KB contiguous rows)
    import ml_dtypes

    out = np.empty((NPAIR, 128, 8, 256), ml_dtypes.bfloat16)
    for m in range(NPAIR):
        sl = slice(m * 128, (m + 1) * 128)
        out[m, :, :, 0:128] = (
            wk[:, sl].reshape(8, 128, 128).transpose(1, 0, 2)
        ).astype(ml_dtypes.bfloat16)
        out[m, :, :, 128:256] = (
            wq[:, sl].reshape(8, 128, 128).transpose(1, 0, 2)
        ).astype(ml_dtypes.bfloat16)
    return np.ascontiguousarray(out)


def _in_maps(x, Wq, bq, Wk, bk, Wv, bv):
    import ml_dtypes

    bf = ml_dtypes.bfloat16
    maps = []
    for c in range(NCORES):
        b, hh = c // 2, c % 2
        cs = slice(hh * COLS, (hh + 1) * COLS)
        maps.append(
            {
                "xT": np.ascontiguousarray(np.asarray(x)[b].T).astype(bf),
                "wkq": _w_slab(np.asarray(Wk)[:, cs], np.asarray(Wq)[:, cs]),
                "wv": np.ascontiguousarray(np.asarray(Wv)[:, cs]).astype(bf),
                "bq": np.ascontiguousarray(np.asarray(bq)[cs]),
                "bk": np.ascontiguousarray(np.asarray(bk)[cs]),
                "bv": np.ascontiguousarray(np.asarray(bv)[cs]),
            }
        )
    return maps


def _run(inputs, trace=False):
    from concourse import bass_utils

    nc = _get_nc()
    res = bass_utils.run_bass_kernel_spmd(
        nc,
        _in_maps(**inputs),
        core_ids=list(range(NCORES)),
        trace=trace,
    )
    out = np.empty((B, S, D), np.float32)
    for c in range(NCORES):
        b, hh = c // 2, c % 2
        out[b, :, hh * COLS : (hh + 1) * COLS] = res.results[c]["out"].astype(
            np.float32
        )
    return out, res


def kernel(**inputs):
    out, _ = _run(inputs, trace=False)
    return out


if __name__ == "__main__":
    _get_nc()
    print("build ok")


# revision 25
# speedup vs baseline: 1.1415x; 1.0513x over previous
"""Trainium2 Bass kernel for nn_AttentionLayer (B=4, S=2048, H=16, DH=64).

Sharding: 8 cores = 4 batches x 2 head-halves. Core c handles batch c//2,
heads (c%2)*8 .. (c%2)*8+8 (i.e. 512 of the 1024 QKV columns).

Per-core device program (SPMD, same program on all cores, different inputs):
  inputs (pre-laid-out on host, x/W in bf16 to halve DMA):
    xT  [1024, 2048] bf16 = x[b].T       (contraction dim on partitions)
    wq/wk/wv [1024, 512] bf16            (column slice for this core's heads)
    bq/bk/bv [512] f32
  output:
    out [2048, 512] f32 = attention ctx, seq-major (q rows, head*64+dh cols);
    host only reshapes.

v3 design (vs baseline, which ran PV as V_aug.T @ E at f32r N=512):
  - QKV projections from bf16 x/W (same modeled PE rate, half the HBM
    traffic; input DMA was starving the exp pipeline at startup).
  - Scores kept [128 kpos, q] f32r (N=512 matmuls, full rate); exp on ACT
    (PSUM->SBUF, bf16 out). ACT is the modeled bottleneck engine:
    256 ops x (1024*0.833 + 185)ns ~ 266us busy, at its floor.
  - PV transposed: ctx[q, dh] += E[:, q128].T @ V_aug[kb], with
    V_aug = [V | ones] bf16 [128, 65]; N=65 at bf16 full rate. Column 64
    accumulates the softmax denominator for free. This is the big PE win:
    PV drops from 512 to 65 cycles per matmul (f32r would be 4x-penalized
    at N<256); total PE ~721k -> ~592k cycles (~300 -> ~247us busy).
  - PSUM zero-region rule (start=True lazily zeroes the whole 2KB bank):
    each unit's ctx lives in TWO banks of [128, 4, 65] with exactly one
    accumulation group per bank (start at kb==0 s==first, stop at kb==15
    s==last). PSUM: scores 2x2 banks + ctx 2x1 + qkv 2 = 8.
  - Normalize straight from PSUM on DVE: reciprocal of the denominator
    column then per-subtile scalar_tensor_tensor (ctx * (1/den)) + bv
    (bv folded post-normalize since sum(p)=1), seq-major DMA out per
    4-subtile half.
"""

import numpy as np

B, S, H, DH = 4, 2048, 16, 64
D = H * DH  # 1024
NCORES = 8
COLS = 512  # qkv columns per core (8 heads)
NPAIR = 4  # head pairs per core
NKB = S // 128  # 16 k-blocks
QG = 1024  # q-group width
NQG = S // QG  # 2
XC = 512  # x streaming chunk (seq cols)
NXC = S // XC  # 4
INV_SQRT_DH = 1.0 / 8.0

_CACHE = {}


def _build():
    import concourse.mybir as mybir
    import concourse.tile as tile
    from concourse import bacc

    f32 = mybir.dt.float32
    f32r = mybir.dt.float32r
    bf16 = mybir.dt.bfloat16
    Exp = mybir.ActivationFunctionType.Exp
    Alu = mybir.AluOpType

    nc = bacc.Bacc(
        "TRN2",
        target_bir_lowering=False,
        debug=False,
        enable_asserts=False,
        num_devices=NCORES,
    )

    xT_d = nc.dram_tensor("xT", [D, S], bf16, kind="ExternalInput").ap()
    # Wk|Wq host-pre-rearranged+packed to SBUF tile layout [128, 8, 256]
    # (wk in cols 0:128, wq in 128:256): 4KB contiguous rows, one full-rate
    # dma_start per pair for both weights
    wkq_d = nc.dram_tensor(
        "wkq", [NPAIR, 128, 8, 256], bf16, kind="ExternalInput"
    ).ap()
    wv_d = nc.dram_tensor("wv", [D, COLS], bf16, kind="ExternalInput").ap()
    bq_d = nc.dram_tensor("bq", [COLS], f32, kind="ExternalInput").ap()
    bk_d = nc.dram_tensor("bk", [COLS], f32, kind="ExternalInput").ap()
    bv_d = nc.dram_tensor("bv", [COLS], f32, kind="ExternalInput").ap()
    out_d = nc.dram_tensor("out", [S, COLS], bf16, kind="ExternalOutput").ap()

    with tile.TileContext(nc) as tc:
        with (
            tc.tile_pool(name="consts", bufs=1) as consts,
            tc.tile_pool(name="vpool", bufs=1) as vpool,
            tc.tile_pool(name="wvpool", bufs=1) as wvpool,
            tc.tile_pool(name="wqk", bufs=2) as wqk,
            tc.tile_pool(name="xpool", bufs=1) as xpool,
            tc.tile_pool(name="qkt", bufs=2) as qkt,
            tc.tile_pool(name="epool", bufs=48) as epool,
            tc.tile_pool(name="opool", bufs=3) as opool,
            tc.tile_pool(name="psum", bufs=1, space="PSUM") as psum,
        ):
            # ---- constants, ACT table preload ----
            bq_t = consts.tile([128, NPAIR], f32)
            bk_t = consts.tile([128, NPAIR], f32)
            bv_s = consts.tile([1, COLS], f32)
            bvb = consts.tile([128, 8, DH], f32)  # bv broadcast per head
            nc.gpsimd.dma_start(out=bq_t, in_=bq_d.rearrange("(m p) -> p m", p=128))
            nc.gpsimd.dma_start(out=bk_t, in_=bk_d.rearrange("(m p) -> p m", p=128))
            nc.gpsimd.dma_start(out=bv_s, in_=bv_d[None, :])
            for h in range(8):
                nc.gpsimd.partition_broadcast(
                    bvb[:, h, :], bv_s[:, h * DH : (h + 1) * DH]
                )
            warm = consts.tile([1, 1], f32)
            nc.vector.memset(warm, 0.0)
            nc.scalar.activation(warm, warm, Exp)  # pull ACT table load early
            # ramp the PE p-state during the initial DMA wait: ~4us of dummy
            # matmuls so the first real projection runs at full clock
            wzero = consts.tile([128, 512], bf16)
            nc.vector.memset(wzero, 0.0)
            wps = psum.tile([128, 512], f32, tag="p1", bufs=2, name="wps")
            for i in range(8):
                nc.tensor.matmul(
                    wps,
                    lhsT=wzero[:, 0:128],
                    rhs=wzero,
                    start=(i == 0),
                    stop=(i == 7),
                )

            # V_aug, bf16: vt[i][:, h, 0:64] = V chunk, col 64 = ones (denom)
            vt = [vpool.tile([128, 8, 65], bf16, name=f"vt{i}") for i in range(NKB)]
            for i in range(NKB):
                nc.vector.memset(vt[i][:, :, 64:65], 1.0)

            wv = wvpool.tile([128, 8, COLS], bf16, name="wv")

            def load_wv():
                nc.sync.dma_start(
                    out=wv, in_=wv_d.rearrange("(j p) c -> p j c", p=128)
                )

            def load_x_chunk(c, fine=False):
                # x[b] in bf16 is only 32KB/partition: stream it ONCE and keep
                # it resident; every head-pair's projections reuse it.
                # 2 half-issues (4 quarter-issues for the startup-critical
                # chunks) so projection chains start on partial data; 8 per-j
                # dma_starts would serialize on HWDGE issue overhead.
                xt = xpool.tile([128, 8, XC], bf16, name=f"xt{c}", tag=f"xt{c}")
                n = 4 if fine else 2
                for hh in range(n):
                    nc.sync.dma_start(
                        out=xt[:, hh * 8 // n : (hh + 1) * 8 // n, :],
                        in_=xT_d[
                            hh * 1024 // n : (hh + 1) * 1024 // n,
                            c * XC : (c + 1) * XC,
                        ].rearrange("(j p) c -> p j c", p=128),
                    )
                return xt

            def load_w_pair(m, defer_dma=False):
                wkqm = wqk.tile([128, 8, 256], bf16, name="wkqm", tag="wkqm")
                if not defer_dma:
                    nc.sync.dma_start(out=wkqm, in_=wkq_d[m])
                return wkqm[:, :, 128:256], wkqm[:, :, 0:128], wkqm

            def qk_chunk(m, c, xt, wqm, wkm, qt, kt, projs=("k", "q")):
                pairs = {"q": (wqm, bq_t, qt), "k": (wkm, bk_t, kt)}
                for w, bias, dst in (pairs[p] for p in projs):
                    ps = psum.tile([128, 512], f32, tag="p1", bufs=2)
                    for j in range(8):
                        nc.tensor.matmul(
                            ps,
                            lhsT=w[:, j, :],
                            rhs=xt[:, j, :],
                            start=(j == 0),
                            stop=(j == 7),
                        )
                    nc.vector.tensor_scalar_add(
                        dst[:, c * XC : (c + 1) * XC], ps, bias[:, m : m + 1]
                    )

            def v_chunk(c, xt, mm):
                # per-PAIR V slice (rhs N=128, same total PE cycles as the
                # 8-head chain): pair m's V is emitted in pair m's own filler
                # window, so pair 0's window is not flooded by all of V
                for i in range(XC // 128):
                    it = c * (XC // 128) + i
                    ps = psum.tile([128, 128], f32, tag="p1", bufs=2)
                    for j in range(8):
                        nc.tensor.matmul(
                            ps,
                            lhsT=xt[:, j, i * 128 : (i + 1) * 128],
                            rhs=wv[:, j, mm * 128 : (mm + 1) * 128],
                            start=(j == 0),
                            stop=(j == 7),
                        )
                    # bv folded in post-normalize (sum p = 1): plain cast
                    # copy (DVE: GPSIMD cannot access PSUM on TRN2)
                    nc.vector.tensor_copy(
                        vt[it][:, 2 * mm : 2 * mm + 2, 0:64],
                        ps.rearrange("p (h d) -> p h d", h=2),
                    )

            def emit_attention_unit(m, q0, p0, head, qt, kt, qw=QG):
                nsub = qw // 128
                nhalf = (nsub + 3) // 4
                ctx = [
                    psum.tile(
                        [128, 4, 65], f32, tag=f"ctx{i}", bufs=1, name=f"ctx{i}"
                    )
                    for i in range(nhalf)
                ]

                def scores(kb):
                    sc = psum.tile([128, QG], f32, tag="sc", bufs=2, name="sc")
                    for qq in range(qw // 512):
                        nc.tensor.matmul(
                            sc[:, qq * 512 : (qq + 1) * 512],
                            lhsT=kt[p0 : p0 + 64, kb * 128 : (kb + 1) * 128],
                            rhs=qt[
                                p0 : p0 + 64,
                                q0 + qq * 512 : q0 + (qq + 1) * 512,
                            ],
                            start=True,
                            stop=True,
                        )
                    return sc

                # software-pipelined emission, two scores ahead (see baseline)
                scs = [scores(0), scores(1)]
                for kb in range(NKB):
                    ee = epool.tile([128, QG], bf16, tag="e", name="ee")
                    nc.scalar.activation(
                        ee[:, 0:qw], scs[kb % 2][:, 0:qw], Exp, scale=INV_SQRT_DH
                    )
                    if kb < NKB - 2:
                        scs[kb % 2] = scores(kb + 2)
                    for s in range(nsub):
                        # one accumulation group per PSUM bank (zero region):
                        # start on the bank's first write, stop on its last
                        nc.tensor.matmul(
                            ctx[s // 4][:, s % 4, :],
                            lhsT=ee[:, s * 128 : (s + 1) * 128],
                            rhs=vt[kb][:, head, :],
                            start=(kb == 0 and s % 4 == 0),
                            stop=(kb == NKB - 1 and s % 4 == 3),
                        )
                # normalize: bulk-copy BOTH ctx banks PSUM->SBUF first (frees
                # them for the next unit's PV asap), then recip + per-subtile
                # (ctx * 1/den) + bv from SBUF, seq-major DMA per half
                css = []
                for i in range(nhalf):
                    cs = opool.tile([128, 4, 65], f32, tag="c", name="cs")
                    nc.vector.tensor_copy(cs, ctx[i])
                    css.append(cs)
                for i, cs in enumerate(css):
                    rr = opool.tile([128, 4, 1], f32, tag="r", name="rr")
                    nc.vector.reciprocal(rr, cs[:, :, 64:65])
                    ob = opool.tile([128, 4, DH], bf16, tag="o", name="ob")
                    for s in range(4):
                        nc.vector.scalar_tensor_tensor(
                            ob[:, s, :],
                            cs[:, s, 0:64],
                            rr[:, s, :],
                            bvb[:, head, :],
                            op0=Alu.mult,
                            op1=Alu.add,
                        )
                    nc.sync.dma_start(
                        out=out_d[
                            q0 + i * 512 : q0 + (i + 1) * 512,
                            head * DH : (head + 1) * DH,
                        ].rearrange("(s p) d -> p s d", p=128),
                        in_=ob,
                    )

            # ---- banded priorities: attention preferred, QKV/V fill gaps ----
            from contextlib import contextmanager

            base = tc.cur_priority + 50
            att_cur = [base]
            fill_cur = [base + 6000]

            @contextmanager
            def band(cursor):
                off = tc.cur_priority - cursor[0]
                with tc.high_priority(offset=off):
                    yield
                    cursor[0] = tc.cur_priority

            # ---- per pair: QKV (filler band) then attention (att band) ----
            xts = {}
            for m in range(NPAIR):
                with band(fill_cur):
                    wqm, wkm, wkqm = load_w_pair(m, defer_dma=(m == 0))
                    qt = qkt.tile([128, S], bf16, name=f"qt{m}", tag="qt")
                    kt = qkt.tile([128, S], bf16, name=f"kt{m}", tag="kt")
                    if m == 0:
                        # all QK chunks first (kt feeds every unit's scores;
                        # exp stalls if kt c2/c3 are late), V strictly after
                        nc.sync.dma_start(out=wkqm, in_=wkq_d[m])
                        xts[0] = load_x_chunk(0, fine=True)
                        xts[1] = load_x_chunk(1, fine=True)
                        load_wv()
                        xts[2] = load_x_chunk(2)
                        xts[3] = load_x_chunk(3)
                        qk_chunk(m, 0, xts[0], wqm, wkm, qt, kt)
                        qk_chunk(m, 1, xts[1], wqm, wkm, qt, kt)
                        qk_chunk(m, 2, xts[2], wqm, wkm, qt, kt)
                        qk_chunk(m, 3, xts[3], wqm, wkm, qt, kt)
                        for c in range(NXC):
                            v_chunk(c, xts[c], m)
                    else:
                        for c in range(NXC):
                            qk_chunk(
                                m, c, xts[c], wqm, wkm, qt, kt,
                                projs=("k", "q") if c < 2 else ("k",),
                            )
                        for c in range(NXC):
                            v_chunk(c, xts[c], m)

                # ---- attention units (Q c2/c3 deferred after qg0) ----
                for qg in range(NQG):
                    q0 = qg * QG
                    for h in range(2):
                        head = 2 * m + h
                        p0 = h * 64
                        with band(att_cur):
                            emit_attention_unit(m, q0, p0, head, qt, kt)
                    if qg == 0 and m > 0:
                        with band(fill_cur):
                            for c in (2, 3):
                                qk_chunk(m, c, xts[c], wqm, wkm, qt, kt, projs=("q",))

    nc.compile()
    return nc


def _get_nc():
    if "nc" not in _CACHE:
        _CACHE["nc"] = _build()
    return _CACHE["nc"]


def _w_slab(wk, wq):
    # [D, 512] x2 -> [NPAIR, 128, 8, 256]: per pair m, wk cols m*128:(m+1)*128
    # at [..., 0:128] and wq's at [..., 128:256], rows (j*128+p) -> [p, j, c]
    # (the SBUF tile layout, 4KB contiguous rows)
    import ml_dtypes

    out = np.empty((NPAIR, 128, 8, 256), ml_dtypes.bfloat16)
    for m in range(NPAIR):
        sl = slice(m * 128, (m + 1) * 128)
        out[m, :, :, 0:128] = (
            wk[:, sl].reshape(8, 128, 128).transpose(1, 0, 2)
        ).astype(ml_dtypes.bfloat16)
        out[m, :, :, 128:256] = (
            wq[:, sl].reshape(8, 128, 128).transpose(1, 0, 2)
        ).astype(ml_dtypes.bfloat16)
    return np.ascontiguousarray(out)


def _in_maps(x, Wq, bq, Wk, bk, Wv, bv):
    import ml_dtypes

    bf = ml_dtypes.bfloat16
    maps = []
    for c in range(NCORES):
        b, hh = c // 2, c % 2
        cs = slice(hh * COLS, (hh + 1) * COLS)
        maps.append(
            {
                "xT": np.ascontiguousarray(np.asarray(x)[b].T).astype(bf),
                "wkq": _w_slab(np.asarray(Wk)[:, cs], np.asarray(Wq)[:, cs]),
                "wv": np.ascontiguousarray(np.asarray(Wv)[:, cs]).astype(bf),
                "bq": np.ascontiguousarray(np.asarray(bq)[cs]),
                "bk": np.ascontiguousarray(np.asarray(bk)[cs]),
                "bv": np.ascontiguousarray(np.asarray(bv)[cs]),
            }
        )
    return maps


def _run(inputs, trace=False):
    from concourse import bass_utils

    nc = _get_nc()
    res = bass_utils.run_bass_kernel_spmd(
        nc,
        _in_maps(**inputs),
        core_ids=list(range(NCORES)),
        trace=trace,
    )
    out = np.empty((B, S, D), np.float32)
    for c in range(NCORES):
        b, hh = c // 2, c % 2
        out[b, :, hh * COLS : (hh + 1) * COLS] = res.results[c]["out"].astype(
            np.float32
        )
    return out, res


def kernel(**inputs):
    out, _ = _run(inputs, trace=False)
    return out


if __name__ == "__main__":
    _get_nc()
    print("build ok")
